# revision 18
# baseline (speedup 1.0000x reference)
"""v8: 2-pair score tiles + merged score matmuls.

Scores for a pair of window-row-pairs (a "tile", 4 window rows) share one
[128, 8, 128] PSUM tile — all 8 slots used (v6/v7 wasted half). Score
matmuls use absolute-chunk-parity row groups (even k-chunks on PE rows
0:63, odd on 64:127) so shared chunks stream both pairs' queries in one
256-col matmul: 5 matmuls per tile instead of 8. exp and mask-mul cover
[128, 1024] once per tile; one [128, 130] PSUM->SBUF cast per tile.

Slot layout per tile (pairs p=2t, p2=p+1; chunk s = key rows 2s,2s+1):
  lo:  slot 0 = (p, c0)  slots 1,2 = chunk p+2 -> (p, c2), (p2, c1)
       slot 3 = (p2, c3)
  hi:  slots 4,5 = chunk p+1 -> (p, c1), (p2, c0)
       slots 6,7 = chunk p+3 -> (p, c3), (p2, c2)
c_of_slot = [0, 2, 1, 3, 1, 0, 3, 2].

Softmax normalization happens on the host: the device emits numerator
and denominator (exp carries bias -4 to keep f16 in range).

See kernel_v6.py / kernel_v7.py for the lineage.
"""

import numpy as np

B, C, H, W, K = 8, 64, 64, 64, 7
HC = WC = H - K + 1          # 58
N = HC * WC                  # 3364
NPAIR = HC // 2              # 29 window-row pairs
NTILE = NPAIR // 2           # 14 full 2-pair tiles (+1 single-pair tail)
SCALE = float(1.0 / np.sqrt(C))
C_OF_SLOT = [0, 2, 1, 3, 1, 0, 3, 2]
SLOT_OF_PAIR_CHUNK = ([0, 4, 1, 6], [5, 2, 7, 3])   # [pair parity][chunk]

_CACHE = {}


def _build_mask_np():
    """[128, 8, 128] band mask, slot order C_OF_SLOT."""
    kk = np.arange(128)[:, None, None]
    c = np.arange(4)[None, :, None]
    col = np.arange(128)[None, None, :]
    k_local = c * 128 + kk
    dI, jp = k_local // W, k_local % W
    jb, j = col // 64, col % 64 - 3
    ok = (j >= 0) & (j < WC) & (dI - jb >= 0) & (dI - jb < K) \
        & (jp - j >= 0) & (jp - j < K)
    m = ok.astype(np.float16)
    m[0, 0, (np.arange(128) % 64 - 3 < 0) | (np.arange(128) % 64 - 3 >= WC)] = 1.0
    return np.ascontiguousarray(m[:, C_OF_SLOT, :])    # [128, 8, 128]


def _build_module():
    import concourse.tile as tile
    from concourse import bacc, mybir

    dt = mybir.dt
    f32 = dt.float32
    f16 = dt.float16

    nc = bacc.Bacc(
        "TRN2", target_bir_lowering=False, debug=False, enable_asserts=False,
        num_devices=8,
    )

    x_d = nc.dram_tensor("x65", [65, H, W], f16, kind="ExternalInput").ap()
    # consts: [128, 1344] f16 = mask[128, 8*128] ++ wqq[65->128, 128] ++
    # wkk[65->128, 128] ++ wv[65->128, 64]  (weight rows 65..127 zero)
    cst_d = nc.dram_tensor("consts", [128, 1344], f16, kind="ExternalInput").ap()
    # out = unnormalized numerator (0:64) ++ softmax denominator (64)
    out_d = nc.dram_tensor("out", [128, NPAIR, C + 1], f16, kind="ExternalOutput").ap()

    with tile.TileContext(nc) as tc:
        with (
            tc.tile_pool(name="const", bufs=1) as const,
            tc.tile_pool(name="qk", bufs=1) as qkpool,
            tc.tile_pool(name="attn", bufs=3) as attnpool,
        ):
            x_sb = const.tile([65, H, W], f16)
            cst_sb = const.tile([128, 1344], f16)
            stage_sb = const.tile([128, NPAIR, C + 1], f16)
            nbias_sb = const.tile([128, 1], f32)
            q_sb = qkpool.tile([128, H, W], f16, tag="q")   # q in both halves
            k_sb = qkpool.tile([128, H, W], f16, tag="k")
            v_sb = qkpool.tile([128, 32, C + 1], f16, tag="v")

            mask_sb = cst_sb[:, 0:1024].rearrange("p (a b) -> p a b", a=8)
            wqq_sb = cst_sb[0:65, 1024:1152]   # [Wq || Wq]
            wkk_sb = cst_sb[0:65, 1152:1280]   # [Wk || Wk]
            wv_sb = cst_sb[0:65, 1280:1344]

            nc.scalar.dma_start(cst_sb[:], cst_d[:])
            nc.sync.dma_start(x_sb[:, 0:32, :], x_d[:, 0:32, :])
            nc.gpsimd.dma_start(x_sb[:, 48:64, :], x_d[:, 48:64, :])
            nc.scalar.dma_start(x_sb[:, 32:48, :], x_d[:, 32:48, :])
            nc.gpsimd.memset(v_sb[:, :, C:C + 1], 1.0)
            nc.gpsimd.memset(nbias_sb[:], -4.0)

            # ---- QKV projections (q/k duplicated via stacked weights) ----
            with (
                tc.tile_pool(name="psqk", bufs=2, space="PSUM") as psqk,
                tc.tile_pool(name="psv", bufs=2, space="PSUM") as psv,
            ):
                for g in range(4):
                    ps = psqk.tile([128, 16, W], f32, tag="ps")
                    for h in range(2):
                        s = 2 * g + h
                        nc.tensor.matmul(
                            ps[:, 8 * h:8 * h + 8, :],
                            wqq_sb,
                            x_sb[:, s * 8:(s + 1) * 8, :],
                        )
                    eng = nc.scalar.copy if g % 2 == 0 else nc.vector.tensor_copy
                    eng(q_sb[:, g * 16:(g + 1) * 16, :], ps[:])
                for g in range(4):
                    ps = psqk.tile([128, 16, W], f32, tag="ps")
                    for h in range(2):
                        s = 2 * g + h
                        nc.tensor.matmul(
                            ps[:, 8 * h:8 * h + 8, :],
                            wkk_sb,
                            x_sb[:, s * 8:(s + 1) * 8, :],
                        )
                    eng = nc.scalar.copy if g % 2 == 1 else nc.vector.tensor_copy
                    eng(k_sb[:, g * 16:(g + 1) * 16, :], ps[:])
                for g in range(8):
                    ps = psv.tile([128, 4, C], f32, tag="psv")
                    for h in range(4):
                        r = 4 * g + h
                        nc.tensor.matmul(
                            ps[:, h, :],
                            x_sb[:, 2 * r:2 * r + 2, :],
                            wv_sb,
                        )
                    eng = nc.scalar.copy if g % 2 == 0 else nc.vector.tensor_copy
                    eng(v_sb[:, 4 * g:4 * g + 4, 0:C], ps[:])

            # ---- banded attention, 2 pairs per tile ----
            with (
                tc.tile_pool(name="pssc", bufs=3, space="PSUM") as pssc,
                tc.tile_pool(name="psout", bufs=2, space="PSUM") as psout,
            ):
                scores = [None] * (NTILE + 1)

                def kch(s, half):
                    base = 64 * half
                    return k_sb[base:base + 64, 2 * s:2 * s + 2, :]

                def emit_scores(t):
                    p = 2 * t
                    i = 2 * p              # first key row of chunk p
                    sc = pssc.tile([128, 8, 128], f32, tag="sc")
                    qlo = q_sb[0:64, :, :]
                    qhi = q_sb[64:128, :, :]
                    if t < NTILE:
                        # lo: even chunks p, p+2, p+4
                        nc.tensor.matmul(sc[:, 0, :], kch(p, 0),
                                         qlo[:, i + 3:i + 5, :])
                        nc.tensor.matmul(sc[:, 1:3, :], kch(p + 2, 0),
                                         qlo[:, i + 3:i + 7, :])
                        nc.tensor.matmul(sc[:, 3, :], kch(p + 4, 0),
                                         qlo[:, i + 5:i + 7, :])
                        # hi: odd chunks p+1, p+3
                        nc.tensor.matmul(sc[:, 4:6, :], kch(p + 1, 1),
                                         qhi[:, i + 3:i + 7, :])
                        nc.tensor.matmul(sc[:, 6:8, :], kch(p + 3, 1),
                                         qhi[:, i + 3:i + 7, :])
                    else:
                        # last single pair 28: chunks 28,30 lo; 29 -> slot 4,
                        # 31 -> slot 6 (c_of_slot matches: 1@4, 3@6)
                        nc.tensor.matmul(sc[:, 0, :], kch(p, 0),
                                         qlo[:, i + 3:i + 5, :])
                        nc.tensor.matmul(sc[:, 1, :], kch(p + 2, 0),
                                         qlo[:, i + 3:i + 5, :])
                        nc.tensor.matmul(sc[:, 4, :], kch(p + 1, 1),
                                         qhi[:, i + 3:i + 5, :])
                        nc.tensor.matmul(sc[:, 6, :], kch(p + 3, 1),
                                         qhi[:, i + 3:i + 5, :])
                    scores[t] = sc

                def emit_tail(t):
                    sc = scores[t]
                    p = 2 * t
                    ex = attnpool.tile([128, 8, 128], f16, tag="ex")
                    at = attnpool.tile([128, 8, 128], f16, tag="at")
                    npair_t = 2 if t < NTILE else 1
                    if t < NTILE:
                        nc.scalar.activation(
                            ex[:], sc[:], mybir.ActivationFunctionType.Exp,
                            scale=SCALE, bias=nbias_sb[:],
                        )
                        nc.vector.tensor_mul(at[:], ex[:], mask_sb)
                    else:
                        nc.scalar.activation(
                            ex[:, 0:2, :], sc[:, 0:2, :],
                            mybir.ActivationFunctionType.Exp,
                            scale=SCALE, bias=nbias_sb[:],
                        )
                        nc.scalar.activation(
                            ex[:, 4:7:2, :], sc[:, 4:7:2, :],
                            mybir.ActivationFunctionType.Exp,
                            scale=SCALE, bias=nbias_sb[:],
                        )
                        nc.vector.tensor_mul(at[:, 0:2, :], ex[:, 0:2, :],
                                             mask_sb[:, 0:2, :])
                        nc.vector.tensor_mul(at[:, 4:7:2, :], ex[:, 4:7:2, :],
                                             mask_sb[:, 4:7:2, :])
                    ops = psout.tile([128, 2, C + 1], f32, tag="ops")
                    for pp in range(npair_t):
                        slots = SLOT_OF_PAIR_CHUNK[pp] if t < NTILE \
                            else SLOT_OF_PAIR_CHUNK[0]
                        for c in range(4):
                            nc.tensor.matmul(
                                ops[:, pp, :],
                                at[:, slots[c], :],
                                v_sb[:, p + pp + c, :],
                                start=(c == 0), stop=(c == 3),
                            )
                    nc.vector.tensor_copy(
                        stage_sb[:, p:p + npair_t, :],
                        ops[:, 0:npair_t, :],
                    )
                    if t in (3, 7, 11, 14):
                        i0 = {3: 0, 7: 8, 11: 16, 14: 24}[t]
                        p1 = p + npair_t
                        nc.sync.dma_start(
                            out_d[:, i0:p1, :], stage_sb[:, i0:p1, :],
                        )

                LAGT = 2
                for t in range(NTILE + 1):
                    emit_scores(t)
                    if t >= LAGT:
                        emit_tail(t - LAGT)
                for t in range(NTILE + 1 - LAGT, NTILE + 1):
                    emit_tail(t)

    nc.compile()
    return nc


def _get_module():
    if "nc" not in _CACHE:
        _CACHE["nc"] = _build_module()
        _CACHE["mask"] = _build_mask_np()
    return _CACHE["nc"], _CACHE["mask"]


def _make_in_maps(x, Wq, bq, Wk, bk, Wv, bv, mask):
    wq65 = np.concatenate([Wq, bq[None]]).astype(np.float16)
    wk65 = np.concatenate([Wk, bk[None]]).astype(np.float16)
    wv65 = np.concatenate([Wv, bv[None]]).astype(np.float16)
    wqq = np.zeros((128, 128), np.float16)
    wqq[0:65, 0:64] = wq65
    wqq[0:65, 64:128] = wq65
    wkk = np.zeros((128, 128), np.float16)
    wkk[0:65, 0:64] = wk65
    wkk[0:65, 64:128] = wk65
    wv = np.zeros((128, 64), np.float16)
    wv[0:65] = wv65
    consts = np.ascontiguousarray(
        np.concatenate([mask.reshape(128, 1024), wqq, wkk, wv], axis=1)
    )
    ones = np.ones((1, H, W), np.float16)
    in_maps = []
    for b in range(B):
        x65 = np.concatenate([np.asarray(x[b]).astype(np.float16), ones])
        in_maps.append({
            "x65": np.ascontiguousarray(x65),
            "consts": consts,
        })
    return in_maps


def _unstage(arr):
    """[128, NPAIR, C+1] f16 num/den staging -> [HC, WC, C] f32."""
    a = arr.astype(np.float32)
    lo = a[3:3 + WC]        # window rows 2i
    hi = a[67:67 + WC]      # window rows 2i+1
    out = np.empty((HC, WC, C), np.float32)
    out[0::2] = (lo[:, :, 0:C] / lo[:, :, C:C + 1]).transpose(1, 0, 2)
    out[1::2] = (hi[:, :, 0:C] / hi[:, :, C:C + 1]).transpose(1, 0, 2)
    return out


def run(inputs, trace=False, **spmd_kwargs):
    from concourse import bass_utils

    nc, mask = _get_module()
    in_maps = _make_in_maps(
        inputs["x"], inputs["Wq"], inputs["bq"], inputs["Wk"], inputs["bk"],
        inputs["Wv"], inputs["bv"], mask,
    )
    res = bass_utils.run_bass_kernel_spmd(
        nc, in_maps, core_ids=list(range(B)), trace=trace, **spmd_kwargs,
    )
    out = np.stack([_unstage(res.results[b]["out"]) for b in range(B)])
    return out, res


def kernel(**inputs) -> np.ndarray:
    return run(inputs)[0]
